# revision 3
# baseline (speedup 1.0000x reference)
"""CrossModalAttention Trainium2 kernel.

Problem: B=2, S=2048 text tokens, NV=2048 visual tokens, model dim 1024,
hidden 512, 16 heads x head_dim 32.

Sharding (8 cores): data-parallel over batch x tensor-parallel over heads.
Core c handles batch (c % 2) and head group (c // 2) -> 4 heads = 128 local
hidden dims. Host slices weights by head group, pre-transposes text/visual to
contraction-major layout, and sums the 4 per-batch partial outputs (the
"all-reduce after the output projection" done at gather time).

Math notes (exact rewrites of the reference):
  - scores = (text@Wq + bq) @ (visual@Wk + bk)^T / sqrt(512); the bk term
    contributes Q@bk which is constant along the softmax axis -> dropped.
  - softmax without max-subtraction: scores have |x| < ~1 here, exp is safe.
  - bv folds out of attention (softmax rows sum to 1):
    attended = P@(visual@Wv) + bv, so out += bv@Wo + bo once, on the host.

Per-core device program (all matmuls in fp32r, ~1e-4 rel):
  phase V (visual): KT = Wk^T @ visualT, VT = Wv^T @ visualT (K=128 c-tiles,
    N=512 chunks), V = PE-transpose(VT) packed as Vaug = [V_h | ones] per head.
  phase Q (text): QT = Wq^T @ textT + bq.
  attention, per s-super of 512 q-tokens, per n-tile of 128 visual tokens:
    - QK^T: 4 row-tiled K=32 matmuls (one per head, tile_position=(32h,0))
      -> scoresT [n=128, s=512] x4 heads in a 4-bank PSUM quad
    - exp: one ScalarE activation over the [128, 2048] quad, scale=1/sqrt(512)
    - EV: 4 plain M=33 matmuls: ev_h[0:33,:] += Vaug_h^T @ ET_h; row 32
      accumulates the softmax denominator.
  per-super epilogue: reciprocal of denominators, broadcast via DRAM-bounce
    DMA, normalize into attT [128d, 512s], fused output projection
    O[s,e] = attT^T @ Wo_local (K=128), DMA out.
"""

import numpy as np

import concourse.bacc as bacc
import concourse.bass as bass
import concourse.mybir as mybir
import concourse.tile as tile
from concourse.bass_utils import run_bass_kernel_spmd
from concourse.masks import make_identity

F32 = mybir.dt.float32
F32R = mybir.dt.float32r

S = 2048          # text tokens
NV = 2048         # visual tokens
C = 1024          # model dim
DL = 128          # local hidden dims (4 heads x 32)
E = 1024          # output dim
NCT = C // 128    # contraction tiles
NSP = 4           # s-supers of 512
SSUP = 512
NNT = NV // 128   # n-tiles
SCALE = float(1.0 / np.sqrt(np.float32(512.0)))

_CACHED_NC = None


def _build_nc():
    nc = bacc.Bacc("TRN2", target_bir_lowering=False, debug=False)

    xt = nc.dram_tensor("xt", [C, S], F32R, kind="ExternalInput")    # textT
    xv = nc.dram_tensor("xv", [C, NV], F32R, kind="ExternalInput")   # visualT
    wq = nc.dram_tensor("wq", [C, DL], F32R, kind="ExternalInput")
    wk = nc.dram_tensor("wk", [C, DL], F32R, kind="ExternalInput")
    wv = nc.dram_tensor("wv", [C, DL], F32R, kind="ExternalInput")
    bq = nc.dram_tensor("bq", [DL], F32, kind="ExternalInput")
    wo = nc.dram_tensor("wo", [DL, E], F32R, kind="ExternalInput")
    out = nc.dram_tensor("o", [S, E], F32, kind="ExternalOutput")
    scratch = nc.dram_tensor("rb_scratch", [4, SSUP], F32)

    with tile.TileContext(nc) as tc, nc.allow_low_precision(
        reason="fp32r matmul pipeline"
    ):
        with (
            tc.tile_pool(name="wpool", bufs=1) as wpool,
            tc.tile_pool(name="stream", bufs=4) as stream,
            tc.tile_pool(name="persist", bufs=1) as persist,
            tc.tile_pool(name="qtp", bufs=4) as qtp,
            tc.tile_pool(name="et", bufs=8) as etp,
            tc.tile_pool(name="attp", bufs=2) as attp,
            tc.tile_pool(name="osb", bufs=3) as osb,
            tc.tile_pool(name="aux", bufs=2) as aux,
        ):
            # ---------------- weights + constants ----------------
            wq_sb = wpool.tile([128, NCT, DL], F32R)
            wk_sb = wpool.tile([128, NCT, DL], F32R)
            wv_sb = wpool.tile([128, NCT, DL], F32R)
            nc.sync.dma_start(out=wq_sb, in_=wq.rearrange("(t p) d -> p t d", p=128))
            nc.sync.dma_start(out=wk_sb, in_=wk.rearrange("(t p) d -> p t d", p=128))
            nc.sync.dma_start(out=wv_sb, in_=wv.rearrange("(t p) d -> p t d", p=128))
            wo_sb = wpool.tile([128, E], F32R)
            nc.sync.dma_start(out=wo_sb, in_=wo[:, :])
            bq_sb = wpool.tile([128, 1], F32)
            nc.sync.dma_start(out=bq_sb, in_=bq.rearrange("(p one) -> p one", one=1))

            ident_f = wpool.tile([128, 128], F32)
            make_identity(nc, ident_f)
            ident = wpool.tile([128, 128], F32R)
            nc.vector.tensor_copy(ident, ident_f)
            ones_f = wpool.tile([128, NNT * 4], F32)
            nc.vector.memset(ones_f, 1.0)

            # persistent activations
            kt_sb = persist.tile([128, NV], F32R)             # KT [d, n]
            vt_sb = persist.tile([128, NV], F32R)             # VT [d, n]
            vaug = persist.tile([128, NNT, 4, 33], F32R)      # [n128, nt, h, d|1]
            recipS = persist.tile([128, SSUP], F32)
            recipB = persist.tile([32, 4, SSUP], F32)

            # ones column of vaug: [128, nt, h, 1] <- ones_f
            nc.vector.tensor_copy(vaug[:, :, :, 32:33], ones_f)

            # ---------------- visual phase: KT, VT ----------------
            with tc.tile_pool(name="pp", bufs=2, space="PSUM") as pp:
                for ch in range(NV // 512):
                    ktp = pp.tile([128, 512], F32, tag="ktp")
                    vtp = pp.tile([128, 512], F32, tag="vtp")
                    for ct in range(NCT):
                        xvt = stream.tile([128, 512], F32R, tag="xv")
                        nc.sync.dma_start(
                            out=xvt,
                            in_=xv[128 * ct:128 * (ct + 1), 512 * ch:512 * (ch + 1)],
                        )
                        nc.tensor.matmul(
                            ktp[:, :], wk_sb[:, ct, :], xvt,
                            start=(ct == 0), stop=(ct == NCT - 1),
                        )
                        nc.tensor.matmul(
                            vtp[:, :], wv_sb[:, ct, :], xvt,
                            start=(ct == 0), stop=(ct == NCT - 1),
                        )
                    nc.vector.tensor_copy(kt_sb[:, 512 * ch:512 * (ch + 1)], ktp)
                    nc.vector.tensor_copy(vt_sb[:, 512 * ch:512 * (ch + 1)], vtp)

                # V = transpose(VT) -> vaug[:, nt, h, 0:32]
                for nt in range(NNT):
                    vxp = pp.tile([128, 128], F32R, tag="vxp")
                    nc.tensor.transpose(
                        vxp[:, :], vt_sb[:, 128 * nt:128 * (nt + 1)], ident
                    )
                    nc.vector.tensor_copy(vaug[:, nt, :, 0:32], vxp)

                # ------------- text phase: QT (+bq) -------------
                qt_tiles = []
                for ch in range(NSP):
                    qtps = pp.tile([128, 512], F32, tag="ktp")
                    for ct in range(NCT):
                        xtt = stream.tile([128, 512], F32R, tag="xv")
                        nc.sync.dma_start(
                            out=xtt,
                            in_=xt[128 * ct:128 * (ct + 1), 512 * ch:512 * (ch + 1)],
                        )
                        nc.tensor.matmul(
                            qtps[:, :], wq_sb[:, ct, :], xtt,
                            start=(ct == 0), stop=(ct == NCT - 1),
                        )
                    qt_sb = qtp.tile([128, 512], F32R, tag=f"qt{ch}")
                    nc.vector.tensor_scalar_add(qt_sb, qtps, bq_sb[:, 0:1])
                    qt_tiles.append(qt_sb)

            # ---------------- attention ----------------
            with tc.tile_pool(name="ap", bufs=1, space="PSUM") as ap:
                for sp in range(NSP):
                    qt_sb = qt_tiles[sp]
                    ev = ap.tile([128, 4, SSUP], F32, tag="ev")
                    for nt in range(NNT):
                        quad = ap.tile([128, 4, SSUP], F32, tag="quad")
                        for h in range(4):
                            nc.tensor.matmul(
                                quad[:, h, :],
                                kt_sb[32 * h:32 * h + 32, 128 * nt:128 * (nt + 1)],
                                qt_sb[32 * h:32 * h + 32, :],
                                start=True, stop=True,
                                tile_position=(32 * h, 0),
                            )
                        et = etp.tile([128, 4, SSUP], F32R, tag="et")
                        nc.scalar.activation(
                            et, quad, mybir.ActivationFunctionType.Exp, scale=SCALE
                        )
                        for h in range(4):
                            nc.tensor.matmul(
                                ev[0:33, h, :],
                                vaug[:, nt, h, :],
                                et[:, h, :],
                                start=(nt == 0), stop=(nt == NNT - 1),
                            )

                    # ---- super epilogue: normalize + output projection ----
                    for h in range(4):
                        nc.vector.reciprocal(
                            recipS[32 * h:32 * h + 1, :], ev[32:33, h, :]
                        )
                    for h in range(4):
                        nc.sync.dma_start(
                            out=scratch[h, :], in_=recipS[32 * h:32 * h + 1, :]
                        )
                    for h in range(4):
                        nc.gpsimd.dma_start(
                            out=recipB[0:32, h, :],
                            in_=bass.AP(
                                tensor=scratch, offset=h * SSUP,
                                ap=[[0, 32], [1, SSUP]],
                            ),
                        )
                    att = attp.tile([128, SSUP], F32R, tag="att")
                    for h in range(4):
                        nc.vector.tensor_mul(
                            att[32 * h:32 * h + 32, :],
                            ev[0:32, h, :],
                            recipB[0:32, h, :],
                        )
                    for cchunk in range(4):
                        op = ap.tile([128, 2, 512], F32, tag="ev")
                        for e in range(2):
                            nc.tensor.matmul(
                                op[:, e, :],
                                att[:, 128 * cchunk:128 * (cchunk + 1)],
                                wo_sb[:, 512 * e:512 * (e + 1)],
                                start=True, stop=True,
                            )
                        o_sb = osb.tile([128, E], F32, tag="o")
                        nc.vector.tensor_copy(
                            o_sb.rearrange("p (e x) -> p e x", e=2), op
                        )
                        nc.sync.dma_start(
                            out=out[512 * sp + 128 * cchunk:
                                    512 * sp + 128 * (cchunk + 1), :],
                            in_=o_sb,
                        )
            _ = aux

    nc.compile()
    return nc


def _get_nc():
    global _CACHED_NC
    if _CACHED_NC is None:
        _CACHED_NC = _build_nc()
    return _CACHED_NC


def kernel(
    text_embeddings, visual_features, Wq, bq, Wk, bk, Wv, bv, Wo, bo,
    _return_raw=False, _trace=False, _tmpdir=None,
):
    text_embeddings = np.asarray(text_embeddings, dtype=np.float32)
    visual_features = np.asarray(visual_features, dtype=np.float32)
    Wq = np.asarray(Wq, dtype=np.float32)
    bq = np.asarray(bq, dtype=np.float32)
    Wk = np.asarray(Wk, dtype=np.float32)
    Wv = np.asarray(Wv, dtype=np.float32)
    bv = np.asarray(bv, dtype=np.float32)
    Wo = np.asarray(Wo, dtype=np.float32)
    bo = np.asarray(bo, dtype=np.float32)

    B = text_embeddings.shape[0]
    assert B == 2 and text_embeddings.shape[1] == S
    # host pre-transpose to contraction-major (c-major) layout
    xts = [np.ascontiguousarray(text_embeddings[b].T) for b in range(B)]
    xvs = [np.ascontiguousarray(visual_features[b].T) for b in range(B)]

    in_maps = []
    for c in range(8):
        b, g = c % 2, c // 2
        sl = slice(128 * g, 128 * (g + 1))
        in_maps.append({
            "xt": xts[b],
            "xv": xvs[b],
            "wq": np.ascontiguousarray(Wq[:, sl]),
            "wk": np.ascontiguousarray(Wk[:, sl]),
            "wv": np.ascontiguousarray(Wv[:, sl]),
            "bq": np.ascontiguousarray(bq[sl]),
            "wo": np.ascontiguousarray(Wo[sl, :]),
        })

    nc = _get_nc()
    res = run_bass_kernel_spmd(
        nc, in_maps, core_ids=list(range(8)), trace=_trace, tmpdir=_tmpdir
    )
    if _return_raw:
        return res

    const = (bv @ Wo + bo).astype(np.float32)  # exact fold of bv/bo
    O = np.empty((B, S, E), dtype=np.float32)
    for b in range(B):
        acc = res.results[b]["o"].copy()
        for g in range(1, 4):
            acc += res.results[2 * g + b]["o"]
        O[b] = acc + const
    return O


# revision 19
# speedup vs baseline: 1.0767x; 1.0767x over previous
"""CrossModalAttention Trainium2 kernel.

Problem: B=2, S=2048 text tokens, NV=2048 visual tokens, model dim 1024,
hidden 512, 16 heads x head_dim 32.

Sharding (8 cores): data-parallel over batch x tensor-parallel over heads.
Core c handles batch (c % 2) and head group (c // 2) -> 4 heads = 128 local
hidden dims. Host slices weights by head group, pre-transposes text/visual to
contraction-major layout, and sums the 4 per-batch partial outputs (the
"all-reduce after the output projection" done at gather time).

Math notes (exact rewrites of the reference):
  - scores = (text@Wq + bq) @ (visual@Wk + bk)^T / sqrt(512); the bk term
    contributes Q@bk which is constant along the softmax axis -> dropped.
  - softmax without max-subtraction: scores here have |x| < ~1, exp is safe.
  - bv folds out of attention (softmax rows sum to 1):
    attended = P@(visual@Wv) + bv, so out += bv@Wo + bo once, on the host.

Per-core device program (matmuls in fp32r, ~1e-4 rel):
  - interleaved prologue: text chunk 0 -> QT[0:512] first, then per visual
    512-chunk: KT/VT projections (K=128 c-tiles), V = PE-transpose(VT) packed
    as Vaug = [V_h | ones], then super-0 attention n-tiles of that chunk.
  - attention, per s-super of 512 q-tokens, per n-tile of 128 visual tokens:
      QK^T: 4 row-tiled K=32 matmuls (tile_position=(32h,0)), one PSUM bank
        per head (bank-aligned: row tiling + unaligned PSUM outputs crashes)
      exp: one ScalarE activation over the [128, 2048] quad, scale=1/sqrt(512)
      EV: 4 plain M=33 matmuls: ev[0:33, h, :] += Vaug_h^T @ ET_h; row 32
        accumulates the softmax denominator. One bank per head.
  - super epilogue (off the ACT critical path): copy unnormalized attT and
    denominator rows out of PSUM (frees banks fast), one partition-parallel
    reciprocal over rows 0..96, denominator broadcast via DRAM-bounce DMA,
    normalize in SBUF, per-head K=32 output projection vs head-major Wo.
"""

import numpy as np

import concourse.bacc as bacc
import concourse.bass as bass
import concourse.mybir as mybir
import concourse.tile as tile
from concourse.bass_utils import run_bass_kernel_spmd
from concourse.masks import make_identity

F32 = mybir.dt.float32
F32R = mybir.dt.float32r

S = 2048          # text tokens
NV = 2048         # visual tokens
C = 1024          # model dim
DL = 128          # local hidden dims (4 heads x 32)
E = 1024          # output dim
NCT = C // 128    # contraction tiles
SSUP = 512
NSP = S // SSUP   # 4 supers
NNT = NV // 128   # n-tiles
SCALE = float(1.0 / np.sqrt(np.float32(512.0)))

_CACHED_NC = None


def _build_nc():
    nc = bacc.Bacc("TRN2", target_bir_lowering=False, debug=False)

    xt = nc.dram_tensor("xt", [C, S], F32R, kind="ExternalInput")    # textT
    xv = nc.dram_tensor("xv", [C, NV], F32R, kind="ExternalInput")   # visualT
    wq = nc.dram_tensor("wq", [C, DL], F32R, kind="ExternalInput")
    wk = nc.dram_tensor("wk", [C, DL], F32R, kind="ExternalInput")
    wv = nc.dram_tensor("wv", [C, DL], F32R, kind="ExternalInput")
    bq = nc.dram_tensor("bq", [DL], F32, kind="ExternalInput")
    wo = nc.dram_tensor("wo", [DL, E], F32R, kind="ExternalInput")
    out = nc.dram_tensor("o", [S, E], F32, kind="ExternalOutput")
    scratch = nc.dram_tensor("rb_scratch", [4, SSUP], F32)

    with tile.TileContext(nc) as tc, nc.allow_low_precision(
        reason="fp32r matmul pipeline"
    ):
        with (
            tc.tile_pool(name="wpool", bufs=1) as wpool,
            tc.tile_pool(name="stream", bufs=8) as stream,
            tc.tile_pool(name="persist", bufs=1) as persist,
            tc.tile_pool(name="qtp", bufs=1) as qtp,
            tc.tile_pool(name="etpool", bufs=6) as etp,
            tc.tile_pool(name="attpool", bufs=1) as attp,
            tc.tile_pool(name="osb", bufs=3) as osb,
        ):
            # ---------------- weights + constants ----------------
            wq_sb = wpool.tile([128, NCT, DL], F32R)
            wk_sb = wpool.tile([128, NCT, DL], F32R)
            wv_sb = wpool.tile([128, NCT, DL], F32R)
            nc.sync.dma_start(out=wq_sb, in_=wq.rearrange("(t p) d -> p t d", p=128))
            nc.sync.dma_start(out=wk_sb, in_=wk.rearrange("(t p) d -> p t d", p=128))
            nc.sync.dma_start(out=wv_sb, in_=wv.rearrange("(t p) d -> p t d", p=128))
            # head-major Wo at partition base 0: [32, h, E]
            woh = wpool.tile([32, 4, E], F32R)
            nc.sync.dma_start(out=woh, in_=wo.rearrange("(h p) e -> p h e", p=32))
            bq_sb = wpool.tile([128, 1], F32)
            nc.sync.dma_start(out=bq_sb, in_=bq.rearrange("(p one) -> p one", one=1))

            ident_f = wpool.tile([128, 128], F32)
            make_identity(nc, ident_f)
            ident = wpool.tile([128, 128], F32R)
            nc.vector.tensor_copy(ident, ident_f)
            ones_f = wpool.tile([128, NNT * 4], F32)
            nc.vector.memset(ones_f, 1.0)

            # persistent activations
            kt_sb = persist.tile([128, NV], F32R)             # KT [d, n]
            vaug = persist.tile([128, NNT, 4, 33], F32R)      # [n128, nt, h, d|1]
            nc.vector.tensor_copy(vaug[:, :, :, 32:33], ones_f)

            qt_tiles = [None] * NSP
            ev_state = {}

            def text_chunk(ch, psum_pool, ptag):
                qtps = psum_pool.tile([128, 512], F32, tag=ptag, name=f"qtps{ch}")
                for ct in range(NCT):
                    xtt = stream.tile(
                        [128, 512], F32R, tag="xv", name=f"xtt{ch}_{ct}"
                    )
                    nc.sync.dma_start(
                        out=xtt,
                        in_=xt[128 * ct:128 * (ct + 1), 512 * ch:512 * (ch + 1)],
                    )
                    nc.tensor.matmul(
                        qtps, wq_sb[:, ct, :], xtt,
                        start=(ct == 0), stop=(ct == NCT - 1),
                    )
                qt_sb = qtp.tile([128, 512], F32R, tag=f"qt{ch}", name=f"qt{ch}")
                nc.vector.tensor_scalar_add(qt_sb, qtps, bq_sb[:, 0:1])
                qt_tiles[ch] = qt_sb

            def visual_chunk(ch, pp):
                ktp = pp.tile([128, 512], F32, tag="ktp", name=f"ktp{ch}")
                vtp = pp.tile([128, 512], F32, tag="vtp", name=f"vtp{ch}")
                for ct in range(NCT):
                    xvt = stream.tile(
                        [128, 512], F32R, tag="xv", name=f"xvt{ch}_{ct}"
                    )
                    nc.sync.dma_start(
                        out=xvt,
                        in_=xv[128 * ct:128 * (ct + 1), 512 * ch:512 * (ch + 1)],
                    )
                    nc.tensor.matmul(
                        ktp[:, :], wk_sb[:, ct, :], xvt,
                        start=(ct == 0), stop=(ct == NCT - 1),
                    )
                    nc.tensor.matmul(
                        vtp[:, :], wv_sb[:, ct, :], xvt,
                        start=(ct == 0), stop=(ct == NCT - 1),
                    )
                nc.vector.tensor_copy(kt_sb[:, 512 * ch:512 * (ch + 1)], ktp)
                vt_sb = attp.tile([128, 512], F32R, tag="vt", name=f"vt{ch}", bufs=2)
                nc.vector.tensor_copy(vt_sb, vtp)
                for j in range(4):
                    nt = 4 * ch + j
                    vxp = pp.tile([128, 128], F32R, tag="ktp", name=f"vxp{nt}")
                    nc.tensor.transpose(
                        vxp[:, :], vt_sb[:, 128 * j:128 * (j + 1)], ident
                    )
                    nc.vector.tensor_copy(vaug[:, nt, :, 0:32], vxp)

            def attn_tile(sp, nt, ap, qd):
                # QK^T quad + exp + EV for one (super, n-tile)
                qt_sb = qt_tiles[sp]
                quad = qd.tile(
                    [128, 4, SSUP], F32, tag="quad", name=f"quad{sp}_{nt}"
                )
                for h in range(4):
                    nc.tensor.matmul(
                        quad[:, h, :],
                        kt_sb[32 * h:32 * h + 32, 128 * nt:128 * (nt + 1)],
                        qt_sb[32 * h:32 * h + 32, :],
                        start=True, stop=True,
                        tile_position=(32 * h, 0),
                    )
                et = etp.tile(
                    [128, 4, SSUP], F32R, tag="et", name=f"et{sp}_{nt}"
                )
                nc.scalar.activation(
                    et.rearrange("p h s -> p (h s)"),
                    quad.rearrange("p h s -> p (h s)"),
                    mybir.ActivationFunctionType.Exp, scale=SCALE,
                )
                ev = ev_state[sp]
                for h in range(4):
                    nc.tensor.matmul(
                        ev[0:33, h, :],
                        vaug[:, nt, h, :],
                        et[:, h, :],
                        start=(nt == 0), stop=(nt == NNT - 1),
                    )

            def super_epilogue(sp, ap):
                ev = ev_state.pop(sp)
                # free the EV banks fast: unnormalized attT + denominator rows
                attU = attp.tile([32, 4, SSUP], F32, tag="attU", name=f"aU{sp}")
                for h in range(4):
                    nc.vector.tensor_copy(attU[0:32, h, :], ev[0:32, h, :])
                recipS = attp.tile([128, SSUP], F32, tag="recipS", name=f"rS{sp}")
                nc.vector.memset(recipS, 1.0)
                for h in range(4):
                    nc.vector.tensor_copy(
                        recipS[32 * h:32 * h + 1, :], ev[32:33, h, :]
                    )
                # one partition-parallel reciprocal across rows 0..96
                recipR = attp.tile([128, SSUP], F32, tag="recipR", name=f"rR{sp}")
                nc.vector.reciprocal(recipR[0:97, :], recipS[0:97, :])
                # broadcast each head's recip row to 32 partitions (DRAM bounce)
                for h in range(4):
                    nc.sync.dma_start(
                        out=scratch[h, :], in_=recipR[32 * h:32 * h + 1, :]
                    )
                recipB = attp.tile([32, 4, SSUP], F32, tag="recipB", name=f"rB{sp}")
                for h in range(4):
                    nc.gpsimd.dma_start(
                        out=recipB[0:32, h, :],
                        in_=bass.AP(
                            tensor=scratch, offset=h * SSUP,
                            ap=[[0, 32], [1, SSUP]],
                        ),
                    )
                attN = attp.tile([32, 4, SSUP], F32R, tag="attN", name=f"aN{sp}")
                for h in range(4):
                    nc.vector.tensor_mul(
                        attN[0:32, h, :], attU[0:32, h, :], recipB[0:32, h, :]
                    )
                # output projection: per-head K=32 accumulation
                op = ap.tile([128, 2, 512], F32, tag="ev", name=f"op{sp}")
                for cchunk in range(4):
                    o_sb = osb.tile(
                        [128, E], F32, tag="o", name=f"o{sp}_{cchunk}"
                    )
                    for e in range(2):
                        for h in range(4):
                            nc.tensor.matmul(
                                op[:, e, :],
                                attN[0:32, h,
                                     128 * cchunk:128 * (cchunk + 1)],
                                woh[0:32, h, 512 * e:512 * (e + 1)],
                                start=(h == 0), stop=(h == 3),
                            )
                    nc.vector.tensor_copy(
                        o_sb.rearrange("p (e x) -> p e x", e=2), op
                    )
                    nc.sync.dma_start(
                        out=out[SSUP * sp + 128 * cchunk:
                                SSUP * sp + 128 * (cchunk + 1), :],
                        in_=o_sb,
                    )

            # ---------------- emission ----------------
            with tc.tile_pool(name="pp", bufs=1, space="PSUM") as pp:
                text_chunk(0, pp, "qacc")
                for ch in range(4):
                    visual_chunk(ch, pp)
            with (
                tc.tile_pool(name="ap", bufs=1, space="PSUM") as ap,
                tc.tile_pool(name="qd", bufs=1, space="PSUM") as qd,
            ):
                for sp in range(NSP):
                    ev_state[sp] = ap.tile(
                        [128, 4, SSUP], F32, tag="ev", name=f"ev_sp{sp}"
                    )
                    for nt in range(NNT):
                        attn_tile(sp, nt, ap, qd)
                    if sp == 0:
                        for ch in range(1, NSP):
                            text_chunk(ch, qd, "quad")
                    super_epilogue(sp, ap)

    nc.compile()
    return nc


def _get_nc():
    global _CACHED_NC
    if _CACHED_NC is None:
        _CACHED_NC = _build_nc()
    return _CACHED_NC


def kernel(
    text_embeddings, visual_features, Wq, bq, Wk, bk, Wv, bv, Wo, bo,
    _return_raw=False, _trace=False, _tmpdir=None,
):
    text_embeddings = np.asarray(text_embeddings, dtype=np.float32)
    visual_features = np.asarray(visual_features, dtype=np.float32)
    Wq = np.asarray(Wq, dtype=np.float32)
    bq = np.asarray(bq, dtype=np.float32)
    Wk = np.asarray(Wk, dtype=np.float32)
    Wv = np.asarray(Wv, dtype=np.float32)
    bv = np.asarray(bv, dtype=np.float32)
    Wo = np.asarray(Wo, dtype=np.float32)
    bo = np.asarray(bo, dtype=np.float32)

    B = text_embeddings.shape[0]
    assert B == 2 and text_embeddings.shape[1] == S
    xts = [np.ascontiguousarray(text_embeddings[b].T) for b in range(B)]
    xvs = [np.ascontiguousarray(visual_features[b].T) for b in range(B)]

    in_maps = []
    for c in range(8):
        b, g = c % 2, c // 2
        sl = slice(128 * g, 128 * (g + 1))
        in_maps.append({
            "xt": xts[b],
            "xv": xvs[b],
            "wq": np.ascontiguousarray(Wq[:, sl]),
            "wk": np.ascontiguousarray(Wk[:, sl]),
            "wv": np.ascontiguousarray(Wv[:, sl]),
            "bq": np.ascontiguousarray(bq[sl]),
            "wo": np.ascontiguousarray(Wo[sl, :]),
        })

    nc = _get_nc()
    res = run_bass_kernel_spmd(
        nc, in_maps, core_ids=list(range(8)), trace=_trace, tmpdir=_tmpdir
    )
    if _return_raw:
        return res

    const = (bv @ Wo + bo).astype(np.float32)  # exact fold of bv/bo
    O = np.empty((B, S, E), dtype=np.float32)
    for b in range(B):
        acc = res.results[b]["o"].copy()
        for g in range(1, 4):
            acc += res.results[2 * g + b]["o"]
        O[b] = acc + const
    return O
